# revision 3
# baseline (speedup 1.0000x reference)
"""Distributed sparse embedding lookup (mean combiner) on 8 Trainium2 cores.

v3 design (data-parallel over output rows, bf16 table replicated):
  - Table uploaded as bf16 rows padded to 256B: block w = 32767 vocab rows
    + 1 zero entry (pad target), so int16 gather indices cover a window and
    pads gather exact zeros. 31 windows.
  - Per window, keys are grouped by output row and laid out in LEVEL runs:
    L0 = first key of every row (rows ordered by descending in-window
    count), Lk = (k+1)-th key of rows with > k keys, each run 128-aligned.
    Because every Lk lists rows in the same order as L0's prefix, folding
    duplicates is a handful of plain slot-range DVE adds:
        L0[0:nk] += Lk   (zeros in pad slots keep this exact).
  - After folding, each window holds one value per distinct row -> ONE
    dma_scatter_add per window (31 scatter instructions total; the cost
    model prices a scatter at a flat ~2.8us regardless of num_idxs).
  - Scatter elem = 128 bf16 units (whole padded entry) into double-wide
    bf16 parity accumulators; the junk half accumulates zeros.
  - Mean division happens once at the end: merged fp32 tile is multiplied
    by per-row reciprocal counts (one broadcast DVE op), then written out
    with a single dense DMA.
"""
import numpy as np
import ml_dtypes

_B, _S, _D = 4096, 26, 64
_V = 1_000_000
_M = 8
_R = _B * _S            # 106496 output rows
_RC = _R // _M          # 13312 rows per core
_VP = _V // 2           # 500000 pair entries
_WIN = 32767            # pair entries per int16 window (+1 zero entry)
_NWIN = (_VP + _WIN - 1) // _WIN     # 16
_ORC = _RC + 128        # +128 pad rows; pads scatter-add into row _RC
_NSLOT = _ORC // 128    # 105 slots (even: 53, odd: 52)
_BG = 1024              # max num_idxs per dma_gather (HW validated)
_BS = 4096              # max num_idxs per dma_scatter_add (HW validated)
_NPAIR = 2              # accumulator pairs

_prog_cache = {}


def _cdiv(a, b):
    return (a + b - 1) // b


def _pack16(v, budget, pad):
    out = np.full(budget, pad, dtype=v.dtype)
    out[: len(v)] = v
    return np.tile(out.reshape(-1, 16).T, (8, 1))


def _prep(values, row_indices):
    """Build per-core level-run layouts.

    Returns (win_meta, in_maps):
      win_meta: per window: dict(wi, lvl_slots=[slots per level], l0_n=max
        true distinct rows across cores, l0_slots, wlen)
      in_maps: per-core dict(gidx, sidx, recip).
    """
    values = np.asarray(values).astype(np.int64)
    row_indices = np.asarray(row_indices).astype(np.int64)
    if np.any(np.diff(row_indices) < 0):
        order = np.argsort(row_indices, kind="stable")
        values, row_indices = values[order], row_indices[order]
    bounds = np.searchsorted(row_indices, np.arange(_M + 1) * _RC)

    # per core, per window: list of levels, each level = (idxs, rows)
    core_levels = []
    recips = []
    for c in range(_M):
        lo, hi = bounds[c], bounds[c + 1]
        keys = values[lo:hi]
        rows = row_indices[lo:hi] - c * _RC
        counts = np.bincount(rows, minlength=_RC).astype(np.float32)
        recip = 1.0 / np.maximum(counts, 1.0)
        # recip laid out [128, _NSLOT]: row r -> partition r%128, slot r//128
        rp = np.ones((128, _NSLOT), np.float32)
        rp[:, :_RC // 128] = recip.reshape(_RC // 128, 128).T
        recips.append(rp)

        pj = keys >> 1
        w = pj // _WIN
        order = np.lexsort((rows, w))
        ks, rs, pjs = keys[order], rows[order], pj[order]
        wb = np.searchsorted(pjs // _WIN, np.arange(_NWIN + 1))
        wins = []
        for wi in range(_NWIN):
            sl = slice(wb[wi], wb[wi + 1])
            k = pjs[sl] - wi * _WIN    # in-window pair idx [0, _WIN)
            par = (ks[sl] & 1).astype(np.float32)
            r = rs[sl]
            # group by row; order rows by (-count, row)
            urow, start, cnt = np.unique(r, return_index=True,
                                         return_counts=True)
            ordr = np.lexsort((urow, -cnt))
            urow, start, cnt = urow[ordr], start[ordr], cnt[ordr]
            cmax = int(cnt.max()) if len(cnt) else 0
            levels = []
            for lv in range(cmax):
                m = cnt > lv
                levels.append((k[start[m] + lv], urow[m],
                               par[start[m] + lv]))
            wins.append(levels)
        core_levels.append(wins)

    win_meta = []
    for wi in range(_NWIN):
        wlen = min(_WIN, _VP - wi * _WIN) + 1    # + zero entry
        nlev = max(len(core_levels[c][wi]) for c in range(_M))
        lvl_n = [max((len(core_levels[c][wi][lv][0])
                      if lv < len(core_levels[c][wi]) else 0)
                     for c in range(_M)) for lv in range(nlev)]
        lvl_n = [max(n, 1) for n in lvl_n]
        lvl_slots = [_cdiv(n, 128) for n in lvl_n]
        win_meta.append(dict(wi=wi, lvl_slots=lvl_slots, lvl_n=lvl_n,
                             l0_n=lvl_n[0], wlen=wlen))

    def _packm(v, budget):
        out = np.zeros(budget, np.float32)
        out[: len(v)] = v
        return out.reshape(-1, 128).T

    in_maps = []
    for c in range(_M):
        g_parts, s_parts, m_parts = [], [], []
        for meta in win_meta:
            wi = meta["wi"]
            zpad = np.int16(meta["wlen"] - 1)    # the zero entry
            levels = core_levels[c][wi]
            for lv, slots in enumerate(meta["lvl_slots"]):
                if lv < len(levels):
                    idxs = levels[lv][0].astype(np.int16)
                    pars = levels[lv][2]
                else:
                    idxs = np.zeros(0, np.int16)
                    pars = np.zeros(0, np.float32)
                g_parts.append(_pack16(idxs, slots * 128, zpad))
                m_parts.append(_packm(pars, slots * 128))
            rows0 = (levels[0][1].astype(np.int16)
                     if levels else np.zeros(0, np.int16))
            s_parts.append(_pack16(rows0, meta["lvl_slots"][0] * 128,
                                   np.int16(_RC)))
        bf = ml_dtypes.bfloat16
        in_maps.append({
            "gidx": np.ascontiguousarray(np.concatenate(g_parts, axis=1)),
            "sidx": np.ascontiguousarray(np.concatenate(s_parts, axis=1)),
            "pmask": np.ascontiguousarray(
                np.concatenate(m_parts, axis=1).astype(bf)),
            "recip": np.ascontiguousarray(recips[c]),
        })
    return win_meta, in_maps


def _build(win_meta, n_reps=1):
    from concourse import bacc, mybir, tile

    nc = bacc.Bacc(None, target_bir_lowering=False, debug=False,
                   num_swdge_queues=1)
    tlen = sum(m["wlen"] for m in win_meta)
    table = nc.dram_tensor("table", [tlen, 2 * _D], mybir.dt.bfloat16,
                           kind="ExternalInput")
    gtot = sum(sum(m["lvl_slots"]) * 8 for m in win_meta)
    stot = sum(m["lvl_slots"][0] * 8 for m in win_meta)
    gidx = nc.dram_tensor("gidx", [128, gtot], mybir.dt.int16,
                          kind="ExternalInput")
    sidx = nc.dram_tensor("sidx", [128, stot], mybir.dt.int16,
                          kind="ExternalInput")
    pmask = nc.dram_tensor("pmask", [128, gtot // 8], mybir.dt.bfloat16,
                          kind="ExternalInput")
    recip = nc.dram_tensor("recip", [128, _NSLOT], mybir.dt.float32,
                           kind="ExternalInput")
    out = nc.dram_tensor("out", [_ORC, _D], mybir.dt.float32,
                         kind="ExternalOutput")
    HGA = (_NSLOT + 1) // 2   # 53
    HGB = _NSLOT // 2         # 52

    with tile.TileContext(nc) as tc:
        with (
            tc.tile_pool(name="acc", bufs=1) as apool,
            tc.tile_pool(name="data", bufs=3) as dpool,
            tc.tile_pool(name="meta", bufs=1) as mpool,
        ):
            accs = []
            for p in range(_NPAIR):
                aa = apool.tile([128, HGA, _D], mybir.dt.bfloat16,
                                tag=f"aa{p}")
                ab = apool.tile([128, HGA, _D], mybir.dt.bfloat16,
                                tag=f"ab{p}")
                nc.vector.memset(aa[:], 0.0)
                nc.vector.memset(ab[:], 0.0)
                accs.append((aa, ab))

            gix = mpool.tile([128, gtot], mybir.dt.int16, tag="gix")
            six = mpool.tile([128, stot], mybir.dt.int16, tag="six")
            rcp = mpool.tile([128, _NSLOT], mybir.dt.float32, tag="rcp")
            pmk = mpool.tile([128, gtot // 8], mybir.dt.bfloat16,
                             tag="pmk")
            nc.sync.dma_start(out=gix[:], in_=gidx[:])
            nc.sync.dma_start(out=six[:], in_=sidx[:])
            nc.sync.dma_start(out=pmk[:], in_=pmask[:])
            nc.sync.dma_start(out=rcp[:], in_=recip[:])

            for _rep in range(n_reps):
                goff = soff = woff = moff = 0
                chain = 0
                for meta in win_meta:
                    lvl_slots = meta["lvl_slots"]
                    wslots = sum(lvl_slots)
                    wa = dpool.tile([128, wslots, 2 * _D],
                                    mybir.dt.bfloat16, tag="wa")
                    # per-level gather slices with TRUE num_idxs: interior
                    # pad indices (level-run tails) are never transferred.
                    lvl_n = meta["lvl_n"]
                    lso = 0
                    for lv, ls in enumerate(lvl_slots):
                        rem = _cdiv(lvl_n[lv], 16) * 16   # 16-wrapped idx tile
                        so = lso
                        while rem > 0:
                            nidx = min(rem, _BG)
                            sn = min(_cdiv(nidx, 128), lso + ls - so)
                            nc.gpsimd.dma_gather(
                                out_ap=wa[:, so:so + sn, :],
                                in_ap=table[woff:woff + meta["wlen"], :],
                                idxs_ap=gix[:, goff + so * 8:
                                            goff + (so + sn) * 8],
                                num_idxs=nidx, num_idxs_reg=nidx,
                                elem_size=2 * _D, queue_num=0,
                            )
                            rem -= nidx
                            so += sn
                        lso += ls
                    # parity extraction: cw = waE + (waO - waE) * pmask
                    cw = dpool.tile([128, wslots, _D], mybir.dt.bfloat16,
                                    tag="cw")
                    nc.vector.tensor_tensor(
                        out=cw[:], in0=wa[:, :, _D:2 * _D],
                        in1=wa[:, :, 0:_D], op=mybir.AluOpType.subtract)
                    nc.vector.tensor_tensor(
                        out=cw[:], in0=cw[:],
                        in1=pmk[:, moff:moff + wslots, None].to_broadcast(
                            [128, wslots, _D]),
                        op=mybir.AluOpType.mult)
                    nc.vector.tensor_tensor(
                        out=cw[:], in0=cw[:], in1=wa[:, :, 0:_D],
                        op=mybir.AluOpType.add)
                    # fold levels into L0 prefix over TRUE extents only
                    # (pad tails beyond lvl_n hold garbage, never read)
                    off = lvl_slots[0]
                    for lv in range(1, len(lvl_slots)):
                        ls = lvl_slots[lv]
                        n = lvl_n[lv]
                        full, rem = n // 128, n % 128
                        if full:
                            nc.vector.tensor_tensor(
                                out=cw[:, 0:full, :], in0=cw[:, 0:full, :],
                                in1=cw[:, off:off + full, :],
                                op=mybir.AluOpType.add,
                            )
                        if rem:
                            nc.vector.tensor_tensor(
                                out=cw[0:rem, full:full + 1, :],
                                in0=cw[0:rem, full:full + 1, :],
                                in1=cw[0:rem, off + full:off + full + 1, :],
                                op=mybir.AluOpType.add,
                            )
                        off += ls
                    # one scatter per window (L0 rows are distinct)
                    n_idx = meta["l0_n"]
                    n_sl = lvl_slots[0]
                    st = 0
                    while n_idx > 0:
                        cur = min(n_idx, _BS)
                        cur_sl = min(_cdiv(cur, 128), n_sl - st)
                        aa, ab = accs[chain % _NPAIR]
                        chain += 1
                        nc.gpsimd.dma_scatter_add(
                            out_ap=aa[:], in_ap=cw[:, st:st + cur_sl, :],
                            idxs_ap=six[:, soff + st * 8:
                                        soff + (st + cur_sl) * 8],
                            num_idxs=cur, num_idxs_reg=cur,
                            elem_size=_D, queue_num=0,
                            sbuf_tokens_per_rank=128,
                            parity_reg=0, out_ap_other=ab[:],
                        )
                        n_idx -= cur
                        st += cur_sl
                    goff += wslots * 8
                    soff += lvl_slots[0] * 8
                    moff += wslots
                    woff += meta["wlen"]

            # merge pairs (real halves only), scale by recip, one dense DMA
            mg = apool.tile([128, _NSLOT, _D], mybir.dt.float32, tag="mg")
            for par in range(2):
                hg = HGA if par == 0 else HGB
                dst = mg[:, par::2, :]
                nc.vector.tensor_tensor(
                    out=dst, in0=accs[0][par][:, :hg, :],
                    in1=accs[1][par][:, :hg, :],
                    op=mybir.AluOpType.add)
                for p in range(2, _NPAIR):
                    nc.vector.tensor_tensor(
                        out=dst, in0=dst, in1=accs[p][par][:, :hg, :],
                        op=mybir.AluOpType.add)
            nc.vector.tensor_tensor(
                out=mg[:], in0=mg[:],
                in1=rcp[:, :, None].to_broadcast([128, _NSLOT, _D]),
                op=mybir.AluOpType.mult)
            out_view = out[:].rearrange("(s p) d -> p s d", p=128)
            nc.sync.dma_start(out=out_view, in_=mg[:])
    nc.compile()
    return nc


def _table_blocks(emb_table):
    t = np.asarray(emb_table, dtype=np.float32).reshape(_VP, 2 * _D)
    bf = ml_dtypes.bfloat16
    blocks = []
    for wi in range(_NWIN):
        lo = wi * _WIN
        hi = min(lo + _WIN, _VP)
        blk = np.zeros((hi - lo + 1, 2 * _D), bf)
        blk[:-1] = t[lo:hi].astype(bf)
        blocks.append(blk)
    return np.ascontiguousarray(np.concatenate(blocks, axis=0))


def _state(values, row_indices, emb_table, n_reps=1):
    win_meta, in_maps = _prep(values, row_indices)
    key = (tuple(tuple(m["lvl_slots"]) + (m["l0_n"],) for m in win_meta),
           n_reps)
    if key not in _prog_cache:
        _prog_cache[key] = _build(win_meta, n_reps=n_reps)
    nc = _prog_cache[key]
    table = _table_blocks(emb_table)
    for m in in_maps:
        m["table"] = table
    return nc, in_maps


def kernel(values, row_indices, emb_table):
    from concourse.bass_utils import run_bass_kernel_spmd

    nc, in_maps = _state(values, row_indices, emb_table)
    res = run_bass_kernel_spmd(nc, in_maps, core_ids=list(range(_M)))
    full = np.concatenate(
        [np.asarray(res.results[c]["out"])[:_RC] for c in range(_M)], axis=0)
    return np.ascontiguousarray(full.reshape(_B, _S, _D))
